# revision 5
# baseline (speedup 1.0000x reference)
"""GCN+JumpingKnowledge distributed Trainium2 kernel (8 NeuronCores).

Strategy: shard destination nodes across 8 cores (6250 each). Per layer:
  - z rows = act^T-chunk @ W on TensorE (direct row layout, no transpose),
    kept in SBUF (z_loc) and written to HBM shard; AllGather -> z_full
    [50000,128] fp16 in Shared DRAM.
  - dma_gather source rows for this core's edges (sorted by dst tile,
    split by src < 32768 for int16 gather indices, padded to a common
    per-(tile,half) block count across cores so one SPMD program fits
    all). Gather calls rotate across the 4 SWDGE queues so descriptor
    generation overlaps across Q7 core pairs.
  - segment-sum via TensorE: psum[feat,dst] += G_blk^T @ S_blk where
    S_blk is HOST-PRECOMPUTED (one-hot x norm, fp16) and streamed from
    HBM - no on-device S construction. Self-loops are a per-tile
    diagonal S block applied to the local z rows (not gathered).
  - BN stats via per-tile ACT accumulators + 1KB AllReduce, fused
    scale/shift/ReLU on ACT; JK max fused; final projection on device.
"""

import os
import sys

import numpy as np

sys.path.insert(0, "/opt/trn_rl_repo")

N = 50000
E = 800000
F = 128
OUTF = 64
N_CORES = 8
SHARD = N // N_CORES  # 6250
TILE = 128
NTILE = (SHARD + TILE - 1) // TILE  # 49
LAST_W = SHARD - (NTILE - 1) * TILE  # 106
HALF = 32768
GRP = 4  # tiles per gather group
BN_EPS = 1e-5
ZCHUNK = 512
MAX_CALL = 1024  # hw limit: idxs per dma_gather call


def _preprocess(edge_index):
    """Host-side edge routing. Returns (structure, per_core_arrays).

    Self-loops are NOT added to the edge lists; they are handled on
    device via a per-tile diagonal S block on the local z rows.
    """
    src = np.asarray(edge_index[0], dtype=np.int64)
    dst = np.asarray(edge_index[1], dtype=np.int64)

    deg = np.bincount(dst, minlength=N).astype(np.float64) + 1.0
    dinv = 1.0 / np.sqrt(deg)

    normval = (dinv[src] * dinv[dst]).astype(np.float32)

    core = dst // SHARD
    tile_id = (dst % SHARD) // TILE
    half = (src >= HALF).astype(np.int64)
    dstoff = ((dst % SHARD) % TILE).astype(np.int64)

    # per (core, tile, half) counts -> cross-core padded block counts
    key = (core * NTILE + tile_id) * 2 + half
    counts = np.bincount(key, minlength=N_CORES * NTILE * 2).reshape(
        N_CORES, NTILE, 2
    )
    maxcnt = counts.max(axis=0)  # [NTILE, 2]
    pad_blocks = np.maximum((maxcnt + TILE - 1) // TILE, 1)  # [NTILE, 2]

    # slot layout: groups of GRP tiles; per group all lo slots then all hi.
    groups = []
    slot_start = np.zeros((NTILE, 2), dtype=np.int64)
    cursor = 0
    for g0 in range(0, NTILE, GRP):
        tiles = list(range(g0, min(g0 + GRP, NTILE)))
        ginfo = {"tiles": tiles}
        for h, nm in ((0, "lo"), (1, "hi")):
            run_slot0 = cursor
            tb = []
            for t in tiles:
                slot_start[t, h] = cursor
                tb.append((cursor, int(pad_blocks[t, h])))
                cursor += int(pad_blocks[t, h]) * TILE
            run_slots = cursor - run_slot0
            calls = []
            o = run_slot0
            while o < run_slot0 + run_slots:
                n = min(MAX_CALL, run_slot0 + run_slots - o)
                calls.append((o, n))
                o += n
            ginfo[nm] = {
                "slot0": run_slot0,
                "nslots": run_slots,
                "tile_blocks": tb,
                "calls": calls,
            }
        groups.append(ginfo)
    total_slots = cursor
    total_blocks = total_slots // TILE

    # S blob layout: per group, per tile: [diag block][lo blocks][hi blocks]
    # column ranges recorded per tile for the device program.
    scol = 0
    s_tile_cols = {}  # tile -> (diag_col0, lo_col0, hi_col0)
    s_group_cols = []  # per group: (col0, ncols)
    for ginfo in groups:
        g_col0 = scol
        for ti, t in enumerate(ginfo["tiles"]):
            diag_c = scol
            scol += TILE
            lo_c = scol
            scol += int(pad_blocks[t, 0]) * TILE
            hi_c = scol
            scol += int(pad_blocks[t, 1]) * TILE
            s_tile_cols[t] = (diag_c, lo_c, hi_c)
        s_group_cols.append((g_col0, scol - g_col0))
    s_total_cols = scol

    # per-core slot content
    per_core = []
    for c in range(N_CORES):
        m = core == c
        e_t = tile_id[m]
        e_h = half[m]
        e_src = src[m]
        e_nv = normval[m]
        e_do = dstoff[m]
        order = np.lexsort((e_h, e_t))
        e_t, e_h = e_t[order], e_h[order]
        e_src, e_nv, e_do = e_src[order], e_nv[order], e_do[order]
        # rank within (t, h) group
        k = e_t * 2 + e_h
        cnt_c = np.bincount(k, minlength=NTILE * 2)
        grp_starts = np.concatenate([[0], np.cumsum(cnt_c)[:-1]])
        rank = np.arange(len(k)) - grp_starts[k]
        slots = slot_start[e_t, e_h] + rank

        idx_vals = np.zeros(total_slots, dtype=np.int16)
        idx_vals[slots] = (e_src - e_h * HALF).astype(np.int16)

        # idx wrapped layout: slot i -> partition i%16 (replicated x8), col i//16
        idx_arr = np.zeros((128, total_slots // 16), dtype=np.int16)
        v16 = idx_vals.reshape(-1, 16).T  # [16, total/16]
        for g in range(8):
            idx_arr[16 * g : 16 * g + 16] = v16

        # S blob [128, s_total_cols] fp16
        sblob = np.zeros((128, s_total_cols), dtype=np.float16)
        # gather-edge entries: block of slot s = s//128, row = s%128,
        # col within block = dstoff
        blk = slots // TILE
        row = slots % TILE
        # block -> S column base: build a map from gather-slot block to S col
        blk_s_col = np.zeros(total_blocks, dtype=np.int64)
        for ginfo in groups:
            for ti, t in enumerate(ginfo["tiles"]):
                diag_c, lo_c, hi_c = s_tile_cols[t]
                s0, nb = ginfo["lo"]["tile_blocks"][ti]
                for j in range(int(pad_blocks[t, 0])):
                    blk_s_col[s0 // TILE + j] = lo_c + j * TILE
                s0, nb = ginfo["hi"]["tile_blocks"][ti]
                for j in range(int(pad_blocks[t, 1])):
                    blk_s_col[s0 // TILE + j] = hi_c + j * TILE
        scols = blk_s_col[blk] + e_do
        sblob[row, scols] = e_nv
        # diag blocks: dinv^2 of each tile's dst nodes
        dinv2 = (dinv * dinv).astype(np.float16)
        for t in range(NTILE):
            diag_c, _, _ = s_tile_cols[t]
            tw = LAST_W if t == NTILE - 1 else TILE
            ii = np.arange(tw)
            sblob[ii, diag_c + ii] = dinv2[c * SHARD + t * TILE + ii]

        per_core.append({"idx": idx_arr, "sblob": sblob})

    structure = {
        "groups": groups,
        "total_slots": total_slots,
        "total_blocks": total_blocks,
        "s_total_cols": s_total_cols,
        "s_tile_cols": s_tile_cols,
        "s_group_cols": s_group_cols,
        "pad_blocks": pad_blocks,
    }
    return structure, per_core


def _build(structure):
    import concourse.bacc as bacc
    import concourse.tile as tile
    from concourse import mybir

    fp32 = mybir.dt.float32
    fp16 = mybir.dt.float16
    i16 = mybir.dt.int16
    AF = mybir.ActivationFunctionType
    OP = mybir.AluOpType

    groups = structure["groups"]
    total_slots = structure["total_slots"]
    s_total_cols = structure["s_total_cols"]
    s_tile_cols = structure["s_tile_cols"]
    s_group_cols = structure["s_group_cols"]
    pad_blocks = structure["pad_blocks"]

    NQ = int(os.environ.get("KGNN_NQ", "4"))

    nc = bacc.Bacc(
        "TRN2", target_bir_lowering=False, num_devices=N_CORES,
        num_swdge_queues=NQ,
    )

    # ---- I/O ----
    xT_in = nc.declare_dram_parameter("xT", [F, SHARD], fp16, isOutput=False)
    idx_in = nc.declare_dram_parameter(
        "idx", [128, total_slots // 16], i16, isOutput=False
    )
    sblob_in = nc.declare_dram_parameter(
        "sblob", [128, s_total_cols], fp16, isOutput=False
    )
    w_in = [
        nc.declare_dram_parameter(f"W{i}", [F, F], fp16, isOutput=False)
        for i in (1, 2, 3)
    ]
    wp_in = nc.declare_dram_parameter("Wp", [F, OUTF], fp16, isOutput=False)
    b_in = [
        nc.declare_dram_parameter(f"b{i}", [F, 1], fp32, isOutput=False)
        for i in (1, 2, 3)
    ]
    bp_in = nc.declare_dram_parameter("bp", [OUTF, 1], fp32, isOutput=False)
    g_in = [
        nc.declare_dram_parameter(f"g{i}", [F, 1], fp32, isOutput=False)
        for i in (1, 2)
    ]
    be_in = [
        nc.declare_dram_parameter(f"be{i}", [F, 1], fp32, isOutput=False)
        for i in (1, 2)
    ]
    out_ext = nc.declare_dram_parameter("outT", [OUTF, SHARD], fp32, isOutput=True)

    with tile.TileContext(nc) as tc:
        from contextlib import ExitStack

        with ExitStack() as ctx:
            dram = ctx.enter_context(tc.tile_pool(name="dram", bufs=1, space="DRAM"))
            singles = ctx.enter_context(tc.tile_pool(name="singles", bufs=1))
            glo_p = ctx.enter_context(tc.tile_pool(name="glo", bufs=2))
            ghi_p = ctx.enter_context(tc.tile_pool(name="ghi", bufs=2))
            s_p = ctx.enter_context(tc.tile_pool(name="spool", bufs=2))
            conv_ps = ctx.enter_context(
                tc.tile_pool(name="convps", bufs=4, space="PSUM")
            )
            z_ps = ctx.enter_context(tc.tile_pool(name="zps", bufs=2, space="PSUM"))
            rstage = ctx.enter_context(tc.tile_pool(name="rstage", bufs=3))
            small = ctx.enter_context(tc.tile_pool(name="small", bufs=2))

            # DRAM internals
            z_shards = [
                dram.tile([SHARD, F], fp16, name=f"z_shard{i}") for i in range(3)
            ]
            z_fulls = [
                dram.tile([N, F], fp16, addr_space="Shared", name=f"z_full{i}")
                for i in range(3)
            ]
            stats_locs = [
                dram.tile([F, 2], fp32, name=f"stats_loc{i}") for i in range(2)
            ]
            stats_globs = [
                dram.tile([F, 2], fp32, addr_space="Shared", name=f"stats_glob{i}")
                for i in range(2)
            ]

            # ---- load constants ----
            idx_sb = singles.tile([128, total_slots // 16], i16)
            nc.sync.dma_start(out=idx_sb[:], in_=idx_in[:])
            w_sb = []
            for i in range(3):
                w = singles.tile([F, F], fp16, name=f"w{i}")
                nc.sync.dma_start(out=w[:], in_=w_in[i][:])
                w_sb.append(w)
            wp_sb = singles.tile([F, OUTF], fp16)
            nc.sync.dma_start(out=wp_sb[:], in_=wp_in[:])
            b_sb = []
            for i in range(3):
                b = singles.tile([F, 1], fp32, name=f"b{i}")
                nc.sync.dma_start(out=b[:], in_=b_in[i][:])
                b_sb.append(b)
            bp_sb = singles.tile([OUTF, 1], fp32)
            nc.sync.dma_start(out=bp_sb[:], in_=bp_in[:])
            g_sb, be_sb = [], []
            for i in range(2):
                g = singles.tile([F, 1], fp32, name=f"g{i}")
                nc.sync.dma_start(out=g[:], in_=g_in[i][:])
                g_sb.append(g)
                be = singles.tile([F, 1], fp32, name=f"be{i}")
                nc.sync.dma_start(out=be[:], in_=be_in[i][:])
                be_sb.append(be)

            # persistent activations
            actA = singles.tile([F, SHARD], fp16)  # layer input act^T
            nc.sync.dma_start(out=actA[:], in_=xT_in[:])
            actB = singles.tile([F, SHARD], fp16)
            conv_sb = singles.tile([F, SHARD], fp32)
            # local z rows, chunked [128 nodes, F] per tile
            z_loc = singles.tile([128, NTILE, F], fp16)
            sumcols = singles.tile([F, NTILE], fp32)
            sqcols = singles.tile([F, NTILE], fp32)
            sq_scratch = singles.tile([F, TILE], fp32)

            qctr = [0]

            def next_q():
                q = qctr[0] % NQ
                qctr[0] += 1
                return q

            def produce_z(act_src):
                """z rows = (act^T chunk)^T @ W per 128-node chunk; keep in
                z_loc and store rows to z_shard; allgather -> z_full."""
                lyr = produce_z.lyr
                z_shard = z_shards[lyr]
                for t in range(NTILE):
                    o = t * TILE
                    tw = LAST_W if t == NTILE - 1 else TILE
                    zp = z_ps.tile([128, F], fp32, tag="zps")
                    nc.tensor.matmul(
                        zp[:tw, :],
                        lhsT=act_src[:, o : o + tw],
                        rhs=w_sb[lyr][:],
                        start=True,
                        stop=True,
                    )
                    nc.scalar.copy(z_loc[:tw, t, :], zp[:tw, :])
                    nc.sync.dma_start(
                        out=z_shard[o : o + tw, :], in_=z_loc[:tw, t, :]
                    )
                nc.gpsimd.collective_compute(
                    "AllGather",
                    mybir.AluOpType.bypass,
                    replica_groups=[list(range(N_CORES))],
                    ins=[z_shard[:].opt()],
                    outs=[z_fulls[lyr][:].opt()],
                )

            def conv_layer(lyr):
                """gather + S-matmul segment sum into conv_sb; bias; stats."""
                z_full = z_fulls[lyr]
                for gi, ginfo in enumerate(groups):
                    # S blocks for this group: one bulk DMA
                    gc0, gnc = s_group_cols[gi]
                    s_sb = s_p.tile([128, gnc], fp16, tag="s")
                    nc.sync.dma_start(out=s_sb[:], in_=sblob_in[:, gc0 : gc0 + gnc])
                    blockmap = {}
                    for nm, run in (("lo", ginfo["lo"]), ("hi", ginfo["hi"])):
                        pool = glo_p if nm == "lo" else ghi_p
                        src_ap = z_full[:, :] if nm == "lo" else z_full[HALF:, :]
                        for (cs0, cns) in run["calls"]:
                            nblk = (cns + TILE - 1) // TILE
                            gbuf = pool.tile([128, nblk, F], fp16, tag=f"g{nm}")
                            nc.gpsimd.dma_gather(
                                gbuf[:],
                                src_ap,
                                idx_sb[:, cs0 // 16 : (cs0 + cns) // 16],
                                cns,
                                cns,
                                F,
                                queue_num=next_q(),
                            )
                            for j in range(nblk):
                                blockmap[cs0 // TILE + j] = (gbuf, j)
                    for ti, t in enumerate(ginfo["tiles"]):
                        diag_c, lo_c, hi_c = s_tile_cols[t]
                        tw = LAST_W if t == NTILE - 1 else TILE
                        cps = conv_ps.tile([F, TILE], fp32, tag="convps")
                        # diag (self-loop) block first
                        nc.tensor.matmul(
                            cps[:],
                            lhsT=z_loc[:, t, :],
                            rhs=s_sb[:, diag_c - gc0 : diag_c - gc0 + TILE],
                            start=True,
                            stop=False,
                        )
                        blocks = []
                        s0, nb = ginfo["lo"]["tile_blocks"][ti]
                        for j in range(nb):
                            gb = s0 // TILE + j
                            blocks.append(blockmap[gb] + (lo_c + j * TILE,))
                        s0, nb = ginfo["hi"]["tile_blocks"][ti]
                        for j in range(nb):
                            gb = s0 // TILE + j
                            blocks.append(blockmap[gb] + (hi_c + j * TILE,))
                        for bi, (gbuf, lb, sc) in enumerate(blocks):
                            nc.tensor.matmul(
                                cps[:],
                                lhsT=gbuf[:, lb, :],
                                rhs=s_sb[:, sc - gc0 : sc - gc0 + TILE],
                                start=False,
                                stop=(bi == len(blocks) - 1),
                            )
                        o = t * TILE
                        nc.scalar.activation(
                            out=conv_sb[:, o : o + tw],
                            in_=cps[:, :tw],
                            func=AF.Identity,
                            bias=b_sb[lyr][:],
                            scale=1.0,
                            accum_out=sumcols[:, t : t + 1],
                        )
                        nc.scalar.activation(
                            out=sq_scratch[:, :tw],
                            in_=conv_sb[:, o : o + tw],
                            func=AF.Square,
                            accum_out=sqcols[:, t : t + 1],
                        )

            def bn_relu(lyr, act_out):
                """global BN stats allreduce + fused scale/shift/relu."""
                ssum = small.tile([F, 1], fp32, tag="ssum")
                nc.vector.tensor_reduce(
                    ssum[:], sumcols[:], axis=mybir.AxisListType.X, op=OP.add
                )
                ssq = small.tile([F, 1], fp32, tag="ssq")
                nc.vector.tensor_reduce(
                    ssq[:], sqcols[:], axis=mybir.AxisListType.X, op=OP.add
                )
                st = small.tile([F, 2], fp32, tag="stats")
                nc.vector.tensor_copy(st[:, 0:1], ssum[:])
                nc.vector.tensor_copy(st[:, 1:2], ssq[:])
                nc.sync.dma_start(out=stats_locs[lyr][:], in_=st[:])
                nc.gpsimd.collective_compute(
                    "AllReduce",
                    OP.add,
                    replica_groups=[list(range(N_CORES))],
                    ins=[stats_locs[lyr][:].opt()],
                    outs=[stats_globs[lyr][:].opt()],
                )
                stg = small.tile([F, 2], fp32, tag="statsg")
                nc.sync.dma_start(out=stg[:], in_=stats_globs[lyr][:])
                mean = small.tile([F, 1], fp32, tag="mean")
                nc.vector.tensor_scalar_mul(mean[:], stg[:, 0:1], 1.0 / N)
                ex2 = small.tile([F, 1], fp32, tag="ex2")
                nc.vector.tensor_scalar_mul(ex2[:], stg[:, 1:2], 1.0 / N)
                var = small.tile([F, 1], fp32, tag="var")
                nc.vector.tensor_tensor(var[:], mean[:], mean[:], op=OP.mult)
                nc.vector.tensor_sub(var[:], ex2[:], var[:])
                nc.vector.tensor_scalar_add(var[:], var[:], BN_EPS)
                std = small.tile([F, 1], fp32, tag="std")
                nc.scalar.sqrt(std[:], var[:])
                rstd = small.tile([F, 1], fp32, tag="rstd")
                nc.vector.reciprocal(rstd[:], std[:])
                scale = small.tile([F, 1], fp32, tag="scale")
                nc.vector.tensor_mul(scale[:], rstd[:], g_sb[lyr][:])
                shift = small.tile([F, 1], fp32, tag="shift")
                nc.vector.tensor_mul(shift[:], mean[:], scale[:])
                nc.vector.tensor_sub(shift[:], be_sb[lyr][:], shift[:])
                nc.scalar.activation(
                    out=act_out[:],
                    in_=conv_sb[:],
                    func=AF.Relu,
                    bias=shift[:],
                    scale=scale[:],
                )

            PHASE = int(os.environ.get("KGNN_PHASE", "9"))
            nc.vector.memset(actB[:], 0.0)
            nc.vector.memset(conv_sb[:], 0.0)
            # ---- layer 1 ----
            if PHASE >= 1:
                produce_z.lyr = 0
                produce_z(actA)
            if PHASE >= 2:
                conv_layer(0)
            if PHASE >= 3:
                bn_relu(0, actB)
            if PHASE >= 4:
                # ---- layer 2 ----
                produce_z.lyr = 1
                produce_z(actB)
                conv_layer(1)
                bn_relu(1, actA)
                # jk12 = max(act1, act2) -> actB
                nc.vector.tensor_max(actB[:], actB[:], actA[:])
            if PHASE >= 5:
                # ---- layer 3 ----
                produce_z.lyr = 2
                produce_z(actA)
                conv_layer(2)
                # conv3 -> fp16 into actA, jk = max(jk12, conv3) -> actB
                nc.scalar.copy(actA[:], conv_sb[:])
                nc.vector.tensor_max(actB[:], actB[:], actA[:])
            # ---- projection ----
            o = 0
            while o < SHARD:
                w = min(ZCHUNK, SHARD - o)
                pp = z_ps.tile([F, ZCHUNK], fp32, tag="zps")
                nc.tensor.matmul(
                    pp[:OUTF, :w], lhsT=wp_sb[:], rhs=actB[:, o : o + w],
                    start=True, stop=True,
                )
                po = rstage.tile([OUTF, ZCHUNK], fp32, tag="pout")
                nc.scalar.activation(
                    out=po[:, :w], in_=pp[:OUTF, :w], func=AF.Identity,
                    bias=bp_sb[:], scale=1.0,
                )
                nc.sync.dma_start(out=out_ext[:, o : o + w], in_=po[:, :w])
                o += w

    nc.compile()
    return nc


_CACHE = {}
_LAST_RESULTS = None


def kernel(**inputs):
    from concourse.bass_utils import run_bass_kernel_spmd

    x = np.asarray(inputs["x"], dtype=np.float32)
    edge_index = np.asarray(inputs["edge_index"])

    ck = hash(edge_index.tobytes())
    if ck not in _CACHE:
        structure, per_core = _preprocess(edge_index)
        nc = _build(structure)
        _CACHE[ck] = (structure, per_core, nc)
    structure, per_core, nc = _CACHE[ck]

    in_maps = []
    for c in range(N_CORES):
        xc = x[c * SHARD : (c + 1) * SHARD].astype(np.float16)
        m = {
            "xT": np.ascontiguousarray(xc.T),
            "idx": per_core[c]["idx"],
            "sblob": per_core[c]["sblob"],
            "W1": np.asarray(inputs["W1"], np.float16),
            "W2": np.asarray(inputs["W2"], np.float16),
            "W3": np.asarray(inputs["W3"], np.float16),
            "Wp": np.asarray(inputs["Wp"], np.float16),
            "b1": np.asarray(inputs["b1"], np.float32).reshape(F, 1),
            "b2": np.asarray(inputs["b2"], np.float32).reshape(F, 1),
            "b3": np.asarray(inputs["b3"], np.float32).reshape(F, 1),
            "bp": np.asarray(inputs["bp"], np.float32).reshape(OUTF, 1),
            "g1": np.asarray(inputs["g1"], np.float32).reshape(F, 1),
            "g2": np.asarray(inputs["g2"], np.float32).reshape(F, 1),
            "be1": np.asarray(inputs["be1"], np.float32).reshape(F, 1),
            "be2": np.asarray(inputs["be2"], np.float32).reshape(F, 1),
        }
        in_maps.append(m)

    trace = os.environ.get("KGNN_TRACE", "0") == "1"
    res = run_bass_kernel_spmd(
        nc, in_maps, core_ids=list(range(N_CORES)), trace=trace
    )
    global _LAST_RESULTS
    _LAST_RESULTS = res
    out = np.empty((N, OUTF), dtype=np.float32)
    for c in range(N_CORES):
        out[c * SHARD : (c + 1) * SHARD] = res.results[c]["outT"].T
    return out


# revision 7
# speedup vs baseline: 1.2511x; 1.2511x over previous
"""GCN+JumpingKnowledge distributed Trainium2 kernel (8 NeuronCores).

Strategy: shard destination nodes across 8 cores (6250 each). Per layer:
  - z rows = act^T-chunk @ W on TensorE (direct row layout, no transpose),
    kept in SBUF (z_loc) and written to HBM shard; AllGather -> z_full
    [50000,128] fp16 in Shared DRAM.
  - dma_gather source rows for this core's edges (sorted by dst tile,
    split by src < 32768 for int16 gather indices, padded to a common
    per-(tile,half) block count across cores so one SPMD program fits
    all). Gather calls rotate across the 4 SWDGE queues so descriptor
    generation overlaps across Q7 core pairs.
  - segment-sum via TensorE: psum[feat,dst] += G_blk^T @ S_blk where
    S_blk is HOST-PRECOMPUTED (one-hot x norm, fp16) and streamed from
    HBM - no on-device S construction. Self-loops are a per-tile
    diagonal S block applied to the local z rows (not gathered).
  - BN stats via per-tile ACT accumulators + 1KB AllReduce, fused
    scale/shift/ReLU on ACT; JK max fused; final projection on device.
"""

import os
import sys

import numpy as np

sys.path.insert(0, "/opt/trn_rl_repo")

N = 50000
E = 800000
F = 128
OUTF = 64
N_CORES = 8
SHARD = N // N_CORES  # 6250
TILE = 128
NTILE = (SHARD + TILE - 1) // TILE  # 49
LAST_W = SHARD - (NTILE - 1) * TILE  # 106
ASPLIT = 24 * TILE  # 3072: shard-local row where region A ends
BSIZE = SHARD - ASPLIT  # 3178
NA = N_CORES * ASPLIT  # 24576 rows in z_fullA
NB = N_CORES * BSIZE  # 25424 rows in z_fullB
GRP = 4  # tiles per gather group
BN_EPS = 1e-5
ZCHUNK = 512
MAX_CALL = 1024  # hw limit: idxs per dma_gather call


def _preprocess(edge_index):
    """Host-side edge routing. Returns (structure, per_core_arrays).

    Self-loops are NOT added to the edge lists; they are handled on
    device via a per-tile diagonal S block on the local z rows.
    """
    src = np.asarray(edge_index[0], dtype=np.int64)
    dst = np.asarray(edge_index[1], dtype=np.int64)

    deg = np.bincount(dst, minlength=N).astype(np.float64) + 1.0
    dinv = 1.0 / np.sqrt(deg)

    normval = (dinv[src] * dinv[dst]).astype(np.float32)

    core = dst // SHARD
    tile_id = (dst % SHARD) // TILE
    src_loc = src % SHARD
    half = (src_loc >= ASPLIT).astype(np.int64)
    # gather index within region A / B layout (per-core half-shards
    # concatenated by the chunked AllGather)
    region_idx = np.where(
        half == 0,
        (src // SHARD) * ASPLIT + src_loc,
        (src // SHARD) * BSIZE + (src_loc - ASPLIT),
    )
    dstoff = ((dst % SHARD) % TILE).astype(np.int64)

    # per (core, tile, half) counts -> cross-core padded block counts
    key = (core * NTILE + tile_id) * 2 + half
    counts = np.bincount(key, minlength=N_CORES * NTILE * 2).reshape(
        N_CORES, NTILE, 2
    )
    maxcnt = counts.max(axis=0)  # [NTILE, 2]
    pad_blocks = np.maximum((maxcnt + TILE - 1) // TILE, 1)  # [NTILE, 2]

    # slot layout: groups of GRP tiles; per group all lo slots then all hi.
    groups = []
    slot_start = np.zeros((NTILE, 2), dtype=np.int64)
    cursor = 0
    for g0 in range(0, NTILE, GRP):
        tiles = list(range(g0, min(g0 + GRP, NTILE)))
        ginfo = {"tiles": tiles}
        for h, nm in ((0, "lo"), (1, "hi")):
            run_slot0 = cursor
            tb = []
            for t in tiles:
                slot_start[t, h] = cursor
                tb.append((cursor, int(pad_blocks[t, h])))
                cursor += int(pad_blocks[t, h]) * TILE
            run_slots = cursor - run_slot0
            calls = []
            o = run_slot0
            while o < run_slot0 + run_slots:
                n = min(MAX_CALL, run_slot0 + run_slots - o)
                calls.append((o, n))
                o += n
            ginfo[nm] = {
                "slot0": run_slot0,
                "nslots": run_slots,
                "tile_blocks": tb,
                "calls": calls,
            }
        groups.append(ginfo)
    total_slots = cursor
    total_blocks = total_slots // TILE

    # S blob layout: per group, per tile: [diag block][lo blocks][hi blocks]
    # column ranges recorded per tile for the device program.
    scol = 0
    s_tile_cols = {}  # tile -> (diag_col0, lo_col0, hi_col0)
    s_group_cols = []  # per group: (col0, ncols)
    for ginfo in groups:
        g_col0 = scol
        for ti, t in enumerate(ginfo["tiles"]):
            diag_c = scol
            scol += TILE
            lo_c = scol
            scol += int(pad_blocks[t, 0]) * TILE
            hi_c = scol
            scol += int(pad_blocks[t, 1]) * TILE
            s_tile_cols[t] = (diag_c, lo_c, hi_c)
        s_group_cols.append((g_col0, scol - g_col0))
    s_total_cols = scol

    # per-core slot content
    per_core = []
    for c in range(N_CORES):
        m = core == c
        e_t = tile_id[m]
        e_h = half[m]
        e_src = region_idx[m]
        e_nv = normval[m]
        e_do = dstoff[m]
        order = np.lexsort((e_h, e_t))
        e_t, e_h = e_t[order], e_h[order]
        e_src, e_nv, e_do = e_src[order], e_nv[order], e_do[order]
        # rank within (t, h) group
        k = e_t * 2 + e_h
        cnt_c = np.bincount(k, minlength=NTILE * 2)
        grp_starts = np.concatenate([[0], np.cumsum(cnt_c)[:-1]])
        rank = np.arange(len(k)) - grp_starts[k]
        slots = slot_start[e_t, e_h] + rank

        idx_vals = np.zeros(total_slots, dtype=np.int16)
        idx_vals[slots] = e_src.astype(np.int16)

        # idx wrapped layout: slot i -> partition i%16 (replicated x8), col i//16
        idx_arr = np.zeros((128, total_slots // 16), dtype=np.int16)
        v16 = idx_vals.reshape(-1, 16).T  # [16, total/16]
        for g in range(8):
            idx_arr[16 * g : 16 * g + 16] = v16

        # S blob [128, s_total_cols] fp16
        sblob = np.zeros((128, s_total_cols), dtype=np.float16)
        # gather-edge entries: block of slot s = s//128, row = s%128,
        # col within block = dstoff
        blk = slots // TILE
        row = slots % TILE
        # block -> S column base: build a map from gather-slot block to S col
        blk_s_col = np.zeros(total_blocks, dtype=np.int64)
        for ginfo in groups:
            for ti, t in enumerate(ginfo["tiles"]):
                diag_c, lo_c, hi_c = s_tile_cols[t]
                s0, nb = ginfo["lo"]["tile_blocks"][ti]
                for j in range(int(pad_blocks[t, 0])):
                    blk_s_col[s0 // TILE + j] = lo_c + j * TILE
                s0, nb = ginfo["hi"]["tile_blocks"][ti]
                for j in range(int(pad_blocks[t, 1])):
                    blk_s_col[s0 // TILE + j] = hi_c + j * TILE
        scols = blk_s_col[blk] + e_do
        sblob[row, scols] = e_nv
        # diag blocks: dinv^2 of each tile's dst nodes
        dinv2 = (dinv * dinv).astype(np.float16)
        for t in range(NTILE):
            diag_c, _, _ = s_tile_cols[t]
            tw = LAST_W if t == NTILE - 1 else TILE
            ii = np.arange(tw)
            sblob[ii, diag_c + ii] = dinv2[c * SHARD + t * TILE + ii]

        per_core.append({"idx": idx_arr, "sblob": sblob})

    structure = {
        "groups": groups,
        "total_slots": total_slots,
        "total_blocks": total_blocks,
        "s_total_cols": s_total_cols,
        "s_tile_cols": s_tile_cols,
        "s_group_cols": s_group_cols,
        "pad_blocks": pad_blocks,
    }
    return structure, per_core


def _build(structure):
    import concourse.bacc as bacc
    import concourse.tile as tile
    from concourse import mybir

    fp32 = mybir.dt.float32
    fp16 = mybir.dt.float16
    i16 = mybir.dt.int16
    AF = mybir.ActivationFunctionType
    OP = mybir.AluOpType

    groups = structure["groups"]
    total_slots = structure["total_slots"]
    s_total_cols = structure["s_total_cols"]
    s_tile_cols = structure["s_tile_cols"]
    s_group_cols = structure["s_group_cols"]
    pad_blocks = structure["pad_blocks"]

    NQ = int(os.environ.get("KGNN_NQ", "4"))

    nc = bacc.Bacc(
        "TRN2", target_bir_lowering=False, num_devices=N_CORES,
        num_swdge_queues=NQ,
    )

    # ---- I/O ----
    xT_in = nc.declare_dram_parameter("xT", [F, SHARD], fp16, isOutput=False)
    idx_in = nc.declare_dram_parameter(
        "idx", [128, total_slots // 16], i16, isOutput=False
    )
    sblob_in = nc.declare_dram_parameter(
        "sblob", [128, s_total_cols], fp16, isOutput=False
    )
    w_in = [
        nc.declare_dram_parameter(f"W{i}", [F, F], fp16, isOutput=False)
        for i in (1, 2, 3)
    ]
    wp_in = nc.declare_dram_parameter("Wp", [F, OUTF], fp16, isOutput=False)
    b_in = [
        nc.declare_dram_parameter(f"b{i}", [F, 1], fp32, isOutput=False)
        for i in (1, 2, 3)
    ]
    bp_in = nc.declare_dram_parameter("bp", [OUTF, 1], fp32, isOutput=False)
    g_in = [
        nc.declare_dram_parameter(f"g{i}", [F, 1], fp32, isOutput=False)
        for i in (1, 2)
    ]
    be_in = [
        nc.declare_dram_parameter(f"be{i}", [F, 1], fp32, isOutput=False)
        for i in (1, 2)
    ]
    out_ext = nc.declare_dram_parameter("outT", [OUTF, SHARD], fp32, isOutput=True)

    with tile.TileContext(nc) as tc:
        from contextlib import ExitStack

        with ExitStack() as ctx:
            dram = ctx.enter_context(tc.tile_pool(name="dram", bufs=1, space="DRAM"))
            singles = ctx.enter_context(tc.tile_pool(name="singles", bufs=1))
            glo_p = ctx.enter_context(tc.tile_pool(name="glo", bufs=3))
            ghi_p = ctx.enter_context(tc.tile_pool(name="ghi", bufs=3))
            s_p = ctx.enter_context(tc.tile_pool(name="spool", bufs=2))
            conv_ps = ctx.enter_context(
                tc.tile_pool(name="convps", bufs=4, space="PSUM")
            )
            z_ps = ctx.enter_context(tc.tile_pool(name="zps", bufs=2, space="PSUM"))
            rstage = ctx.enter_context(tc.tile_pool(name="rstage", bufs=3))
            small = ctx.enter_context(tc.tile_pool(name="small", bufs=2))

            # DRAM internals
            z_shardAs = [
                dram.tile([ASPLIT, F], fp16, name=f"z_shardA{i}") for i in range(3)
            ]
            z_shardBs = [
                dram.tile([BSIZE, F], fp16, name=f"z_shardB{i}") for i in range(3)
            ]
            z_fullAs = [
                dram.tile([NA, F], fp16, addr_space="Shared", name=f"z_fullA{i}")
                for i in range(3)
            ]
            z_fullBs = [
                dram.tile([NB, F], fp16, addr_space="Shared", name=f"z_fullB{i}")
                for i in range(3)
            ]
            stats_locs = [
                dram.tile([F, 2], fp32, name=f"stats_loc{i}") for i in range(2)
            ]
            stats_globs = [
                dram.tile([F, 2], fp32, addr_space="Shared", name=f"stats_glob{i}")
                for i in range(2)
            ]

            # ---- load constants ----
            idx_sb = singles.tile([128, total_slots // 16], i16)
            nc.sync.dma_start(out=idx_sb[:], in_=idx_in[:])
            w_sb = []
            for i in range(3):
                w = singles.tile([F, F], fp16, name=f"w{i}")
                nc.sync.dma_start(out=w[:], in_=w_in[i][:])
                w_sb.append(w)
            wp_sb = singles.tile([F, OUTF], fp16)
            nc.sync.dma_start(out=wp_sb[:], in_=wp_in[:])
            b_sb = []
            for i in range(3):
                b = singles.tile([F, 1], fp32, name=f"b{i}")
                nc.sync.dma_start(out=b[:], in_=b_in[i][:])
                b_sb.append(b)
            bp_sb = singles.tile([OUTF, 1], fp32)
            nc.sync.dma_start(out=bp_sb[:], in_=bp_in[:])
            g_sb, be_sb = [], []
            for i in range(2):
                g = singles.tile([F, 1], fp32, name=f"g{i}")
                nc.sync.dma_start(out=g[:], in_=g_in[i][:])
                g_sb.append(g)
                be = singles.tile([F, 1], fp32, name=f"be{i}")
                nc.sync.dma_start(out=be[:], in_=be_in[i][:])
                be_sb.append(be)

            # persistent activations
            actA = singles.tile([F, SHARD], fp16)  # layer input act^T
            nc.sync.dma_start(out=actA[:], in_=xT_in[:])
            actB = singles.tile([F, SHARD], fp16)
            conv_sb = singles.tile([F, SHARD], fp32)
            # local z rows, chunked [128 nodes, F] per tile
            z_loc = singles.tile([128, NTILE, F], fp16)
            sumcols = singles.tile([F, NTILE], fp32)
            sqcols = singles.tile([F, NTILE], fp32)
            sq_scratch = singles.tile([F, TILE], fp32)

            qctr = [0]

            def next_q():
                q = qctr[0] % NQ
                qctr[0] += 1
                return q

            def produce_z(act_src):
                """z rows = (act^T chunk)^T @ W per 128-node chunk; keep in
                z_loc and store rows to z_shard; allgather -> z_full."""
                lyr = produce_z.lyr
                for t in range(NTILE):
                    o = t * TILE
                    tw = LAST_W if t == NTILE - 1 else TILE
                    zp = z_ps.tile([128, F], fp32, tag="zps")
                    nc.tensor.matmul(
                        zp[:tw, :],
                        lhsT=act_src[:, o : o + tw],
                        rhs=w_sb[lyr][:],
                        start=True,
                        stop=True,
                    )
                    nc.scalar.copy(z_loc[:tw, t, :], zp[:tw, :])
                    if t < 24:
                        nc.sync.dma_start(
                            out=z_shardAs[lyr][o : o + tw, :],
                            in_=z_loc[:tw, t, :],
                        )
                    else:
                        ob = o - ASPLIT
                        nc.sync.dma_start(
                            out=z_shardBs[lyr][ob : ob + tw, :],
                            in_=z_loc[:tw, t, :],
                        )
                    if t == 23:
                        # region A complete: fire its AllGather so lo
                        # gathers overlap the rest of produce_z + AG B
                        nc.gpsimd.collective_compute(
                            "AllGather",
                            mybir.AluOpType.bypass,
                            replica_groups=[list(range(N_CORES))],
                            ins=[z_shardAs[lyr][:].opt()],
                            outs=[z_fullAs[lyr][:].opt()],
                        )
                nc.gpsimd.collective_compute(
                    "AllGather",
                    mybir.AluOpType.bypass,
                    replica_groups=[list(range(N_CORES))],
                    ins=[z_shardBs[lyr][:].opt()],
                    outs=[z_fullBs[lyr][:].opt()],
                )

            def conv_layer(lyr):
                """gather + S-matmul segment sum into conv_sb; bias; stats."""
                z_fullA = z_fullAs[lyr]
                z_fullB = z_fullBs[lyr]
                for gi, ginfo in enumerate(groups):
                    # S blocks for this group: one bulk DMA
                    gc0, gnc = s_group_cols[gi]
                    s_sb = s_p.tile([128, gnc], fp16, tag="s")
                    nc.sync.dma_start(out=s_sb[:], in_=sblob_in[:, gc0 : gc0 + gnc])
                    blockmap = {}
                    for nm, run in (("lo", ginfo["lo"]), ("hi", ginfo["hi"])):
                        pool = glo_p if nm == "lo" else ghi_p
                        src_ap = z_fullA[:, :] if nm == "lo" else z_fullB[:, :]
                        for (cs0, cns) in run["calls"]:
                            nblk = (cns + TILE - 1) // TILE
                            gbuf = pool.tile([128, nblk, F], fp16, tag=f"g{nm}")
                            nc.gpsimd.dma_gather(
                                gbuf[:],
                                src_ap,
                                idx_sb[:, cs0 // 16 : (cs0 + cns) // 16],
                                cns,
                                cns,
                                F,
                                queue_num=next_q(),
                            )
                            for j in range(nblk):
                                blockmap[cs0 // TILE + j] = (gbuf, j)
                    for ti, t in enumerate(ginfo["tiles"]):
                        diag_c, lo_c, hi_c = s_tile_cols[t]
                        tw = LAST_W if t == NTILE - 1 else TILE
                        cps = conv_ps.tile([F, TILE], fp32, tag="convps")
                        # diag (self-loop) block first
                        nc.tensor.matmul(
                            cps[:],
                            lhsT=z_loc[:, t, :],
                            rhs=s_sb[:, diag_c - gc0 : diag_c - gc0 + TILE],
                            start=True,
                            stop=False,
                        )
                        blocks = []
                        s0, nb = ginfo["lo"]["tile_blocks"][ti]
                        for j in range(nb):
                            gb = s0 // TILE + j
                            blocks.append(blockmap[gb] + (lo_c + j * TILE,))
                        s0, nb = ginfo["hi"]["tile_blocks"][ti]
                        for j in range(nb):
                            gb = s0 // TILE + j
                            blocks.append(blockmap[gb] + (hi_c + j * TILE,))
                        for bi, (gbuf, lb, sc) in enumerate(blocks):
                            nc.tensor.matmul(
                                cps[:],
                                lhsT=gbuf[:, lb, :],
                                rhs=s_sb[:, sc - gc0 : sc - gc0 + TILE],
                                start=False,
                                stop=(bi == len(blocks) - 1),
                            )
                        o = t * TILE
                        nc.scalar.activation(
                            out=conv_sb[:, o : o + tw],
                            in_=cps[:, :tw],
                            func=AF.Identity,
                            bias=b_sb[lyr][:],
                            scale=1.0,
                            accum_out=sumcols[:, t : t + 1],
                        )
                        nc.scalar.activation(
                            out=sq_scratch[:, :tw],
                            in_=conv_sb[:, o : o + tw],
                            func=AF.Square,
                            accum_out=sqcols[:, t : t + 1],
                        )

            def bn_relu(lyr, act_out):
                """global BN stats allreduce + fused scale/shift/relu."""
                ssum = small.tile([F, 1], fp32, tag="ssum")
                nc.vector.tensor_reduce(
                    ssum[:], sumcols[:], axis=mybir.AxisListType.X, op=OP.add
                )
                ssq = small.tile([F, 1], fp32, tag="ssq")
                nc.vector.tensor_reduce(
                    ssq[:], sqcols[:], axis=mybir.AxisListType.X, op=OP.add
                )
                st = small.tile([F, 2], fp32, tag="stats")
                nc.vector.tensor_copy(st[:, 0:1], ssum[:])
                nc.vector.tensor_copy(st[:, 1:2], ssq[:])
                nc.sync.dma_start(out=stats_locs[lyr][:], in_=st[:])
                nc.gpsimd.collective_compute(
                    "AllReduce",
                    OP.add,
                    replica_groups=[list(range(N_CORES))],
                    ins=[stats_locs[lyr][:].opt()],
                    outs=[stats_globs[lyr][:].opt()],
                )
                stg = small.tile([F, 2], fp32, tag="statsg")
                nc.sync.dma_start(out=stg[:], in_=stats_globs[lyr][:])
                mean = small.tile([F, 1], fp32, tag="mean")
                nc.vector.tensor_scalar_mul(mean[:], stg[:, 0:1], 1.0 / N)
                ex2 = small.tile([F, 1], fp32, tag="ex2")
                nc.vector.tensor_scalar_mul(ex2[:], stg[:, 1:2], 1.0 / N)
                var = small.tile([F, 1], fp32, tag="var")
                nc.vector.tensor_tensor(var[:], mean[:], mean[:], op=OP.mult)
                nc.vector.tensor_sub(var[:], ex2[:], var[:])
                nc.vector.tensor_scalar_add(var[:], var[:], BN_EPS)
                std = small.tile([F, 1], fp32, tag="std")
                nc.scalar.sqrt(std[:], var[:])
                rstd = small.tile([F, 1], fp32, tag="rstd")
                nc.vector.reciprocal(rstd[:], std[:])
                scale = small.tile([F, 1], fp32, tag="scale")
                nc.vector.tensor_mul(scale[:], rstd[:], g_sb[lyr][:])
                shift = small.tile([F, 1], fp32, tag="shift")
                nc.vector.tensor_mul(shift[:], mean[:], scale[:])
                nc.vector.tensor_sub(shift[:], be_sb[lyr][:], shift[:])
                nc.scalar.activation(
                    out=act_out[:],
                    in_=conv_sb[:],
                    func=AF.Relu,
                    bias=shift[:],
                    scale=scale[:],
                )

            PHASE = int(os.environ.get("KGNN_PHASE", "9"))
            nc.vector.memset(actB[:], 0.0)
            nc.vector.memset(conv_sb[:], 0.0)
            # last z tile is 106 wide; zero the never-written partitions so
            # the diag matmul (which reads all 128) sees finite values
            nc.vector.memset(z_loc[:], 0.0)
            # ---- layer 1 ----
            if PHASE >= 1:
                produce_z.lyr = 0
                produce_z(actA)
            if PHASE >= 2:
                conv_layer(0)
            if PHASE >= 3:
                bn_relu(0, actB)
            if PHASE >= 4:
                # ---- layer 2 ----
                produce_z.lyr = 1
                produce_z(actB)
                conv_layer(1)
                bn_relu(1, actA)
                # jk12 = max(act1, act2) -> actB
                nc.vector.tensor_max(actB[:], actB[:], actA[:])
            if PHASE >= 5:
                # ---- layer 3 ----
                produce_z.lyr = 2
                produce_z(actA)
                conv_layer(2)
                # conv3 -> fp16 into actA, jk = max(jk12, conv3) -> actB
                nc.scalar.copy(actA[:], conv_sb[:])
                nc.vector.tensor_max(actB[:], actB[:], actA[:])
            # ---- projection ----
            o = 0
            while o < SHARD:
                w = min(ZCHUNK, SHARD - o)
                pp = z_ps.tile([F, ZCHUNK], fp32, tag="zps")
                nc.tensor.matmul(
                    pp[:OUTF, :w], lhsT=wp_sb[:], rhs=actB[:, o : o + w],
                    start=True, stop=True,
                )
                po = rstage.tile([OUTF, ZCHUNK], fp32, tag="pout")
                nc.scalar.activation(
                    out=po[:, :w], in_=pp[:OUTF, :w], func=AF.Identity,
                    bias=bp_sb[:], scale=1.0,
                )
                nc.sync.dma_start(out=out_ext[:, o : o + w], in_=po[:, :w])
                o += w

    nc.compile()
    return nc


_CACHE = {}
_LAST_RESULTS = None


def kernel(**inputs):
    from concourse.bass_utils import run_bass_kernel_spmd

    x = np.asarray(inputs["x"], dtype=np.float32)
    edge_index = np.asarray(inputs["edge_index"])

    ck = hash(edge_index.tobytes())
    if ck not in _CACHE:
        structure, per_core = _preprocess(edge_index)
        nc = _build(structure)
        _CACHE[ck] = (structure, per_core, nc)
    structure, per_core, nc = _CACHE[ck]

    in_maps = []
    for c in range(N_CORES):
        xc = x[c * SHARD : (c + 1) * SHARD].astype(np.float16)
        m = {
            "xT": np.ascontiguousarray(xc.T),
            "idx": per_core[c]["idx"],
            "sblob": per_core[c]["sblob"],
            "W1": np.asarray(inputs["W1"], np.float16),
            "W2": np.asarray(inputs["W2"], np.float16),
            "W3": np.asarray(inputs["W3"], np.float16),
            "Wp": np.asarray(inputs["Wp"], np.float16),
            "b1": np.asarray(inputs["b1"], np.float32).reshape(F, 1),
            "b2": np.asarray(inputs["b2"], np.float32).reshape(F, 1),
            "b3": np.asarray(inputs["b3"], np.float32).reshape(F, 1),
            "bp": np.asarray(inputs["bp"], np.float32).reshape(OUTF, 1),
            "g1": np.asarray(inputs["g1"], np.float32).reshape(F, 1),
            "g2": np.asarray(inputs["g2"], np.float32).reshape(F, 1),
            "be1": np.asarray(inputs["be1"], np.float32).reshape(F, 1),
            "be2": np.asarray(inputs["be2"], np.float32).reshape(F, 1),
        }
        in_maps.append(m)

    trace = os.environ.get("KGNN_TRACE", "0") == "1"
    res = run_bass_kernel_spmd(
        nc, in_maps, core_ids=list(range(N_CORES)), trace=trace
    )
    global _LAST_RESULTS
    _LAST_RESULTS = res
    out = np.empty((N, OUTF), dtype=np.float32)
    for c in range(N_CORES):
        out[c * SHARD : (c + 1) * SHARD] = res.results[c]["outT"].T
    return out


# revision 9
# speedup vs baseline: 1.4481x; 1.1575x over previous
"""GCN+JumpingKnowledge distributed Trainium2 kernel (8 NeuronCores).

Strategy: shard destination nodes across 8 cores (6250 each). Per layer:
  - z rows = act^T-chunk @ W on TensorE (direct row layout, no transpose),
    kept in SBUF (z_loc) and written to HBM shard; AllGather -> z_full
    [50000,128] fp16 in Shared DRAM.
  - dma_gather source rows for this core's edges (sorted by dst tile,
    split by src < 32768 for int16 gather indices, padded to a common
    per-(tile,half) block count across cores so one SPMD program fits
    all). Gather calls rotate across the 4 SWDGE queues so descriptor
    generation overlaps across Q7 core pairs.
  - segment-sum via TensorE: psum[feat,dst] += G_blk^T @ S_blk where
    S_blk is HOST-PRECOMPUTED (one-hot x norm, fp16) and streamed from
    HBM - no on-device S construction. Self-loops are a per-tile
    diagonal S block applied to the local z rows (not gathered).
  - BN stats via per-tile ACT accumulators + 1KB AllReduce, fused
    scale/shift/ReLU on ACT; JK max fused; final projection on device.
"""

import os
import sys

import numpy as np

sys.path.insert(0, "/opt/trn_rl_repo")

N = 50000
E = 800000
F = 128
OUTF = 64
N_CORES = 8
SHARD = N // N_CORES  # 6250
TILE = 128
NTILE = (SHARD + TILE - 1) // TILE  # 49
LAST_W = SHARD - (NTILE - 1) * TILE  # 106
ASPLIT = 24 * TILE  # 3072: shard-local row where region A ends
BSIZE = SHARD - ASPLIT  # 3178
NA = N_CORES * ASPLIT  # 24576 rows in z_fullA
NB = N_CORES * BSIZE  # 25424 rows in z_fullB
GRP = 4  # tiles per gather group
BN_EPS = 1e-5
ZCHUNK = 512
MAX_CALL = 1024  # hw limit: idxs per dma_gather call


def _preprocess(edge_index):
    """Host-side edge routing. Returns (structure, per_core_arrays).

    Self-loops are NOT added to the edge lists; they are handled on
    device via a per-tile diagonal S block on the local z rows.
    """
    src = np.asarray(edge_index[0], dtype=np.int64)
    dst = np.asarray(edge_index[1], dtype=np.int64)

    deg = np.bincount(dst, minlength=N).astype(np.float64) + 1.0
    dinv = 1.0 / np.sqrt(deg)

    normval = (dinv[src] * dinv[dst]).astype(np.float32)

    core = dst // SHARD
    tile_id = (dst % SHARD) // TILE
    src_loc = src % SHARD
    half = (src_loc >= ASPLIT).astype(np.int64)
    # gather index within region A / B layout (per-core half-shards
    # concatenated by the chunked AllGather)
    region_idx = np.where(
        half == 0,
        (src // SHARD) * ASPLIT + src_loc,
        (src // SHARD) * BSIZE + (src_loc - ASPLIT),
    )
    dstoff = ((dst % SHARD) % TILE).astype(np.int64)

    # per (core, tile, half) counts -> cross-core padded block counts
    key = (core * NTILE + tile_id) * 2 + half
    counts = np.bincount(key, minlength=N_CORES * NTILE * 2).reshape(
        N_CORES, NTILE, 2
    )
    maxcnt = counts.max(axis=0)  # [NTILE, 2]
    pad_blocks = np.maximum((maxcnt + TILE - 1) // TILE, 1)  # [NTILE, 2]

    # slot layout: groups of GRP tiles; per group all lo slots then all hi.
    groups = []
    slot_start = np.zeros((NTILE, 2), dtype=np.int64)
    cursor = 0
    for g0 in range(0, NTILE, GRP):
        tiles = list(range(g0, min(g0 + GRP, NTILE)))
        ginfo = {"tiles": tiles}
        for h, nm in ((0, "lo"), (1, "hi")):
            run_slot0 = cursor
            tb = []
            for t in tiles:
                slot_start[t, h] = cursor
                tb.append((cursor, int(pad_blocks[t, h])))
                cursor += int(pad_blocks[t, h]) * TILE
            run_slots = cursor - run_slot0
            calls = []
            o = run_slot0
            while o < run_slot0 + run_slots:
                n = min(MAX_CALL, run_slot0 + run_slots - o)
                calls.append((o, n))
                o += n
            ginfo[nm] = {
                "slot0": run_slot0,
                "nslots": run_slots,
                "tile_blocks": tb,
                "calls": calls,
            }
        groups.append(ginfo)
    total_slots = cursor
    total_blocks = total_slots // TILE

    # S blob layout: per group, per tile: [diag block][lo blocks][hi blocks]
    # column ranges recorded per tile for the device program.
    scol = 0
    s_tile_cols = {}  # tile -> (diag_col0, lo_col0, hi_col0)
    s_group_cols = []  # per group: (col0, ncols)
    for ginfo in groups:
        g_col0 = scol
        for ti, t in enumerate(ginfo["tiles"]):
            diag_c = scol
            scol += TILE
            lo_c = scol
            scol += int(pad_blocks[t, 0]) * TILE
            hi_c = scol
            scol += int(pad_blocks[t, 1]) * TILE
            s_tile_cols[t] = (diag_c, lo_c, hi_c)
        s_group_cols.append((g_col0, scol - g_col0))
    s_total_cols = scol

    # per-core slot content
    per_core = []
    for c in range(N_CORES):
        m = core == c
        e_t = tile_id[m]
        e_h = half[m]
        e_src = region_idx[m]
        e_nv = normval[m]
        e_do = dstoff[m]
        order = np.lexsort((e_h, e_t))
        e_t, e_h = e_t[order], e_h[order]
        e_src, e_nv, e_do = e_src[order], e_nv[order], e_do[order]
        # rank within (t, h) group
        k = e_t * 2 + e_h
        cnt_c = np.bincount(k, minlength=NTILE * 2)
        grp_starts = np.concatenate([[0], np.cumsum(cnt_c)[:-1]])
        rank = np.arange(len(k)) - grp_starts[k]
        slots = slot_start[e_t, e_h] + rank

        idx_vals = np.zeros(total_slots, dtype=np.int16)
        idx_vals[slots] = e_src.astype(np.int16)

        # idx wrapped layout: slot i -> partition i%16 (replicated x8), col i//16
        idx_arr = np.zeros((128, total_slots // 16), dtype=np.int16)
        v16 = idx_vals.reshape(-1, 16).T  # [16, total/16]
        for g in range(8):
            idx_arr[16 * g : 16 * g + 16] = v16

        # S blob [128, s_total_cols] fp16
        sblob = np.zeros((128, s_total_cols), dtype=np.float16)
        # gather-edge entries: block of slot s = s//128, row = s%128,
        # col within block = dstoff
        blk = slots // TILE
        row = slots % TILE
        # block -> S column base: build a map from gather-slot block to S col
        blk_s_col = np.zeros(total_blocks, dtype=np.int64)
        for ginfo in groups:
            for ti, t in enumerate(ginfo["tiles"]):
                diag_c, lo_c, hi_c = s_tile_cols[t]
                s0, nb = ginfo["lo"]["tile_blocks"][ti]
                for j in range(int(pad_blocks[t, 0])):
                    blk_s_col[s0 // TILE + j] = lo_c + j * TILE
                s0, nb = ginfo["hi"]["tile_blocks"][ti]
                for j in range(int(pad_blocks[t, 1])):
                    blk_s_col[s0 // TILE + j] = hi_c + j * TILE
        scols = blk_s_col[blk] + e_do
        sblob[row, scols] = e_nv
        # diag blocks: dinv^2 of each tile's dst nodes
        dinv2 = (dinv * dinv).astype(np.float16)
        for t in range(NTILE):
            diag_c, _, _ = s_tile_cols[t]
            tw = LAST_W if t == NTILE - 1 else TILE
            ii = np.arange(tw)
            sblob[ii, diag_c + ii] = dinv2[c * SHARD + t * TILE + ii]

        per_core.append({"idx": idx_arr, "sblob": sblob})

    structure = {
        "groups": groups,
        "total_slots": total_slots,
        "total_blocks": total_blocks,
        "s_total_cols": s_total_cols,
        "s_tile_cols": s_tile_cols,
        "s_group_cols": s_group_cols,
        "pad_blocks": pad_blocks,
    }
    return structure, per_core


def _build(structure):
    import concourse.bacc as bacc
    import concourse.tile as tile
    from concourse import mybir

    fp32 = mybir.dt.float32
    fp16 = mybir.dt.float16
    i16 = mybir.dt.int16
    AF = mybir.ActivationFunctionType
    OP = mybir.AluOpType

    groups = structure["groups"]
    total_slots = structure["total_slots"]
    s_total_cols = structure["s_total_cols"]
    s_tile_cols = structure["s_tile_cols"]
    s_group_cols = structure["s_group_cols"]
    pad_blocks = structure["pad_blocks"]

    NQ = int(os.environ.get("KGNN_NQ", "4"))

    nc = bacc.Bacc(
        "TRN2", target_bir_lowering=False, num_devices=N_CORES,
        num_swdge_queues=NQ,
    )

    # ---- I/O ----
    xT_in = nc.declare_dram_parameter("xT", [F, SHARD], fp16, isOutput=False)
    idx_in = nc.declare_dram_parameter(
        "idx", [128, total_slots // 16], i16, isOutput=False
    )
    sblob_in = nc.declare_dram_parameter(
        "sblob", [128, s_total_cols], fp16, isOutput=False
    )
    w_in = [
        nc.declare_dram_parameter(f"W{i}", [F, F], fp16, isOutput=False)
        for i in (1, 2, 3)
    ]
    wp_in = nc.declare_dram_parameter("Wp", [F, OUTF], fp16, isOutput=False)
    b_in = [
        nc.declare_dram_parameter(f"b{i}", [F, 1], fp32, isOutput=False)
        for i in (1, 2, 3)
    ]
    bp_in = nc.declare_dram_parameter("bp", [OUTF, 1], fp32, isOutput=False)
    g_in = [
        nc.declare_dram_parameter(f"g{i}", [F, 1], fp32, isOutput=False)
        for i in (1, 2)
    ]
    be_in = [
        nc.declare_dram_parameter(f"be{i}", [F, 1], fp32, isOutput=False)
        for i in (1, 2)
    ]
    out_ext = nc.declare_dram_parameter("outT", [OUTF, SHARD], fp32, isOutput=True)

    with tile.TileContext(nc) as tc:
        from contextlib import ExitStack

        with ExitStack() as ctx:
            dram = ctx.enter_context(tc.tile_pool(name="dram", bufs=1, space="DRAM"))
            singles = ctx.enter_context(tc.tile_pool(name="singles", bufs=1))
            glo_p = ctx.enter_context(tc.tile_pool(name="glo", bufs=12))
            ghi_p = ctx.enter_context(tc.tile_pool(name="ghi", bufs=8))
            s_p = ctx.enter_context(tc.tile_pool(name="spool", bufs=2))
            conv_ps = ctx.enter_context(
                tc.tile_pool(name="convps", bufs=4, space="PSUM")
            )
            z_ps = ctx.enter_context(tc.tile_pool(name="zps", bufs=2, space="PSUM"))
            rstage = ctx.enter_context(tc.tile_pool(name="rstage", bufs=3))
            small = ctx.enter_context(tc.tile_pool(name="small", bufs=2))

            # DRAM internals
            z_shardAs = [
                dram.tile([ASPLIT, F], fp16, name=f"z_shardA{i}") for i in range(3)
            ]
            z_shardBs = [
                dram.tile([BSIZE, F], fp16, name=f"z_shardB{i}") for i in range(3)
            ]
            z_fullAs = [
                dram.tile([NA, F], fp16, addr_space="Shared", name=f"z_fullA{i}")
                for i in range(3)
            ]
            z_fullBs = [
                dram.tile([NB, F], fp16, addr_space="Shared", name=f"z_fullB{i}")
                for i in range(3)
            ]
            stats_locs = [
                dram.tile([F, 2], fp32, name=f"stats_loc{i}") for i in range(2)
            ]
            stats_globs = [
                dram.tile([F, 2], fp32, addr_space="Shared", name=f"stats_glob{i}")
                for i in range(2)
            ]

            # ---- load constants ----
            idx_sb = singles.tile([128, total_slots // 16], i16)
            nc.sync.dma_start(out=idx_sb[:], in_=idx_in[:])
            w_sb = []
            for i in range(3):
                w = singles.tile([F, F], fp16, name=f"w{i}")
                nc.sync.dma_start(out=w[:], in_=w_in[i][:])
                w_sb.append(w)
            wp_sb = singles.tile([F, OUTF], fp16)
            nc.sync.dma_start(out=wp_sb[:], in_=wp_in[:])
            b_sb = []
            for i in range(3):
                b = singles.tile([F, 1], fp32, name=f"b{i}")
                nc.sync.dma_start(out=b[:], in_=b_in[i][:])
                b_sb.append(b)
            bp_sb = singles.tile([OUTF, 1], fp32)
            nc.sync.dma_start(out=bp_sb[:], in_=bp_in[:])
            g_sb, be_sb = [], []
            for i in range(2):
                g = singles.tile([F, 1], fp32, name=f"g{i}")
                nc.sync.dma_start(out=g[:], in_=g_in[i][:])
                g_sb.append(g)
                be = singles.tile([F, 1], fp32, name=f"be{i}")
                nc.sync.dma_start(out=be[:], in_=be_in[i][:])
                be_sb.append(be)

            # persistent activations
            actA = singles.tile([F, SHARD], fp16)  # layer input act^T
            nc.sync.dma_start(out=actA[:], in_=xT_in[:])
            actB = singles.tile([F, SHARD], fp16)
            conv_sb = singles.tile([F, SHARD], fp32)
            # local z rows, chunked [128 nodes, F] per tile
            z_loc = singles.tile([128, NTILE, F], fp16)
            sumcols = singles.tile([F, NTILE], fp32)
            sqcols = singles.tile([F, NTILE], fp32)
            sq_scratch = singles.tile([F, TILE], fp32)

            qctr = [0]

            def next_q():
                q = qctr[0] % NQ
                qctr[0] += 1
                return q

            def produce_z(act_src):
                """z rows = (act^T chunk)^T @ W per 128-node chunk; keep in
                z_loc and store rows to z_shard; allgather -> z_full."""
                lyr = produce_z.lyr
                for t in range(NTILE):
                    o = t * TILE
                    tw = LAST_W if t == NTILE - 1 else TILE
                    zp = z_ps.tile([128, F], fp32, tag="zps")
                    nc.tensor.matmul(
                        zp[:tw, :],
                        lhsT=act_src[:, o : o + tw],
                        rhs=w_sb[lyr][:],
                        start=True,
                        stop=True,
                    )
                    nc.scalar.copy(z_loc[:tw, t, :], zp[:tw, :])
                    if t < 24:
                        nc.sync.dma_start(
                            out=z_shardAs[lyr][o : o + tw, :],
                            in_=z_loc[:tw, t, :],
                        )
                    else:
                        ob = o - ASPLIT
                        nc.sync.dma_start(
                            out=z_shardBs[lyr][ob : ob + tw, :],
                            in_=z_loc[:tw, t, :],
                        )
                    if t == 23:
                        # region A complete: fire its AllGather so lo
                        # gathers overlap the rest of produce_z + AG B
                        nc.gpsimd.collective_compute(
                            "AllGather",
                            mybir.AluOpType.bypass,
                            replica_groups=[list(range(N_CORES))],
                            ins=[z_shardAs[lyr][:].opt()],
                            outs=[z_fullAs[lyr][:].opt()],
                        )
                nc.gpsimd.collective_compute(
                    "AllGather",
                    mybir.AluOpType.bypass,
                    replica_groups=[list(range(N_CORES))],
                    ins=[z_shardBs[lyr][:].opt()],
                    outs=[z_fullBs[lyr][:].opt()],
                )

            def conv_layer(lyr):
                """gather + S-matmul segment sum into conv_sb; bias; stats."""
                z_fullA = z_fullAs[lyr]
                z_fullB = z_fullBs[lyr]

                def issue_run(nm, ginfo, blockmap):
                    run = ginfo[nm]
                    pool = glo_p if nm == "lo" else ghi_p
                    src_ap = z_fullA[:, :] if nm == "lo" else z_fullB[:, :]
                    for (cs0, cns) in run["calls"]:
                        nblk = (cns + TILE - 1) // TILE
                        gbuf = pool.tile([128, nblk, F], fp16, tag=f"g{nm}")
                        nc.gpsimd.dma_gather(
                            gbuf[:],
                            src_ap,
                            idx_sb[:, cs0 // 16 : (cs0 + cns) // 16],
                            cns,
                            cns,
                            F,
                            queue_num=next_q(),
                        )
                        for j in range(nblk):
                            blockmap[cs0 // TILE + j] = (gbuf, j)

                # lo gathers lead one group ahead of hi gathers + convs so
                # the hi run's AllGather-B wait never head-of-line blocks
                # the next group's lo descriptors on the GpSimd engine.
                blockmaps = {}
                for gi, ginfo in enumerate(groups):
                    blockmaps[gi] = {}
                    issue_run("lo", ginfo, blockmaps[gi])
                    if gi > 0:
                        issue_run("hi", groups[gi - 1], blockmaps[gi - 1])
                        do_group(lyr, gi - 1, blockmaps[gi - 1])
                issue_run("hi", groups[-1], blockmaps[len(groups) - 1])
                do_group(lyr, len(groups) - 1, blockmaps[len(groups) - 1])

            def do_group(lyr, gi, blockmap):
                ginfo = groups[gi]
                gc0, gnc = s_group_cols[gi]
                s_sb = s_p.tile([128, gnc], fp16, tag="s")
                nc.sync.dma_start(out=s_sb[:], in_=sblob_in[:, gc0 : gc0 + gnc])
                if True:
                    for ti, t in enumerate(ginfo["tiles"]):
                        diag_c, lo_c, hi_c = s_tile_cols[t]
                        tw = LAST_W if t == NTILE - 1 else TILE
                        cps = conv_ps.tile([F, TILE], fp32, tag="convps")
                        # diag (self-loop) block first
                        nc.tensor.matmul(
                            cps[:],
                            lhsT=z_loc[:, t, :],
                            rhs=s_sb[:, diag_c - gc0 : diag_c - gc0 + TILE],
                            start=True,
                            stop=False,
                        )
                        blocks = []
                        s0, nb = ginfo["lo"]["tile_blocks"][ti]
                        for j in range(nb):
                            gb = s0 // TILE + j
                            blocks.append(blockmap[gb] + (lo_c + j * TILE,))
                        s0, nb = ginfo["hi"]["tile_blocks"][ti]
                        for j in range(nb):
                            gb = s0 // TILE + j
                            blocks.append(blockmap[gb] + (hi_c + j * TILE,))
                        for bi, (gbuf, lb, sc) in enumerate(blocks):
                            nc.tensor.matmul(
                                cps[:],
                                lhsT=gbuf[:, lb, :],
                                rhs=s_sb[:, sc - gc0 : sc - gc0 + TILE],
                                start=False,
                                stop=(bi == len(blocks) - 1),
                            )
                        o = t * TILE
                        nc.scalar.activation(
                            out=conv_sb[:, o : o + tw],
                            in_=cps[:, :tw],
                            func=AF.Identity,
                            bias=b_sb[lyr][:],
                            scale=1.0,
                            accum_out=sumcols[:, t : t + 1],
                        )
                        nc.scalar.activation(
                            out=sq_scratch[:, :tw],
                            in_=conv_sb[:, o : o + tw],
                            func=AF.Square,
                            accum_out=sqcols[:, t : t + 1],
                        )

            def bn_relu(lyr, act_out):
                """global BN stats allreduce + fused scale/shift/relu."""
                ssum = small.tile([F, 1], fp32, tag="ssum")
                nc.vector.tensor_reduce(
                    ssum[:], sumcols[:], axis=mybir.AxisListType.X, op=OP.add
                )
                ssq = small.tile([F, 1], fp32, tag="ssq")
                nc.vector.tensor_reduce(
                    ssq[:], sqcols[:], axis=mybir.AxisListType.X, op=OP.add
                )
                st = small.tile([F, 2], fp32, tag="stats")
                nc.vector.tensor_copy(st[:, 0:1], ssum[:])
                nc.vector.tensor_copy(st[:, 1:2], ssq[:])
                nc.sync.dma_start(out=stats_locs[lyr][:], in_=st[:])
                nc.gpsimd.collective_compute(
                    "AllReduce",
                    OP.add,
                    replica_groups=[list(range(N_CORES))],
                    ins=[stats_locs[lyr][:].opt()],
                    outs=[stats_globs[lyr][:].opt()],
                )
                stg = small.tile([F, 2], fp32, tag="statsg")
                nc.sync.dma_start(out=stg[:], in_=stats_globs[lyr][:])
                mean = small.tile([F, 1], fp32, tag="mean")
                nc.vector.tensor_scalar_mul(mean[:], stg[:, 0:1], 1.0 / N)
                ex2 = small.tile([F, 1], fp32, tag="ex2")
                nc.vector.tensor_scalar_mul(ex2[:], stg[:, 1:2], 1.0 / N)
                var = small.tile([F, 1], fp32, tag="var")
                nc.vector.tensor_tensor(var[:], mean[:], mean[:], op=OP.mult)
                nc.vector.tensor_sub(var[:], ex2[:], var[:])
                nc.vector.tensor_scalar_add(var[:], var[:], BN_EPS)
                std = small.tile([F, 1], fp32, tag="std")
                nc.scalar.sqrt(std[:], var[:])
                rstd = small.tile([F, 1], fp32, tag="rstd")
                nc.vector.reciprocal(rstd[:], std[:])
                scale = small.tile([F, 1], fp32, tag="scale")
                nc.vector.tensor_mul(scale[:], rstd[:], g_sb[lyr][:])
                shift = small.tile([F, 1], fp32, tag="shift")
                nc.vector.tensor_mul(shift[:], mean[:], scale[:])
                nc.vector.tensor_sub(shift[:], be_sb[lyr][:], shift[:])
                nc.scalar.activation(
                    out=act_out[:],
                    in_=conv_sb[:],
                    func=AF.Relu,
                    bias=shift[:],
                    scale=scale[:],
                )

            PHASE = int(os.environ.get("KGNN_PHASE", "9"))
            nc.vector.memset(actB[:], 0.0)
            nc.vector.memset(conv_sb[:], 0.0)
            # last z tile is 106 wide; zero the never-written partitions so
            # the diag matmul (which reads all 128) sees finite values
            nc.vector.memset(z_loc[:], 0.0)
            # ---- layer 1 ----
            if PHASE >= 1:
                produce_z.lyr = 0
                produce_z(actA)
            if PHASE >= 2:
                conv_layer(0)
            if PHASE >= 3:
                bn_relu(0, actB)
            if PHASE >= 4:
                # ---- layer 2 ----
                produce_z.lyr = 1
                produce_z(actB)
                conv_layer(1)
                bn_relu(1, actA)
                # jk12 = max(act1, act2) -> actB
                nc.vector.tensor_max(actB[:], actB[:], actA[:])
            if PHASE >= 5:
                # ---- layer 3 ----
                produce_z.lyr = 2
                produce_z(actA)
                conv_layer(2)
                # conv3 -> fp16 into actA, jk = max(jk12, conv3) -> actB
                nc.scalar.copy(actA[:], conv_sb[:])
                nc.vector.tensor_max(actB[:], actB[:], actA[:])
            # ---- projection ----
            o = 0
            while o < SHARD:
                w = min(ZCHUNK, SHARD - o)
                pp = z_ps.tile([F, ZCHUNK], fp32, tag="zps")
                nc.tensor.matmul(
                    pp[:OUTF, :w], lhsT=wp_sb[:], rhs=actB[:, o : o + w],
                    start=True, stop=True,
                )
                po = rstage.tile([OUTF, ZCHUNK], fp32, tag="pout")
                nc.scalar.activation(
                    out=po[:, :w], in_=pp[:OUTF, :w], func=AF.Identity,
                    bias=bp_sb[:], scale=1.0,
                )
                nc.sync.dma_start(out=out_ext[:, o : o + w], in_=po[:, :w])
                o += w

    nc.compile()
    return nc


_CACHE = {}
_LAST_RESULTS = None


def kernel(**inputs):
    from concourse.bass_utils import run_bass_kernel_spmd

    x = np.asarray(inputs["x"], dtype=np.float32)
    edge_index = np.asarray(inputs["edge_index"])

    ck = hash(edge_index.tobytes())
    if ck not in _CACHE:
        structure, per_core = _preprocess(edge_index)
        nc = _build(structure)
        _CACHE[ck] = (structure, per_core, nc)
    structure, per_core, nc = _CACHE[ck]

    in_maps = []
    for c in range(N_CORES):
        xc = x[c * SHARD : (c + 1) * SHARD].astype(np.float16)
        m = {
            "xT": np.ascontiguousarray(xc.T),
            "idx": per_core[c]["idx"],
            "sblob": per_core[c]["sblob"],
            "W1": np.asarray(inputs["W1"], np.float16),
            "W2": np.asarray(inputs["W2"], np.float16),
            "W3": np.asarray(inputs["W3"], np.float16),
            "Wp": np.asarray(inputs["Wp"], np.float16),
            "b1": np.asarray(inputs["b1"], np.float32).reshape(F, 1),
            "b2": np.asarray(inputs["b2"], np.float32).reshape(F, 1),
            "b3": np.asarray(inputs["b3"], np.float32).reshape(F, 1),
            "bp": np.asarray(inputs["bp"], np.float32).reshape(OUTF, 1),
            "g1": np.asarray(inputs["g1"], np.float32).reshape(F, 1),
            "g2": np.asarray(inputs["g2"], np.float32).reshape(F, 1),
            "be1": np.asarray(inputs["be1"], np.float32).reshape(F, 1),
            "be2": np.asarray(inputs["be2"], np.float32).reshape(F, 1),
        }
        in_maps.append(m)

    trace = os.environ.get("KGNN_TRACE", "0") == "1"
    res = run_bass_kernel_spmd(
        nc, in_maps, core_ids=list(range(N_CORES)), trace=trace
    )
    global _LAST_RESULTS
    _LAST_RESULTS = res
    out = np.empty((N, OUTF), dtype=np.float32)
    for c in range(N_CORES):
        out[c * SHARD : (c + 1) * SHARD] = res.results[c]["outT"].T
    return out
